# revision 9
# baseline (speedup 1.0000x reference)
"""AASIST Trainium2 kernel: 8-core data-parallel (2 samples/core).

Strategy:
- Host: compute sinc filters, fold BN into conv weights/biases, fold SELU's
  lambda into downstream weights, pack all weights (bf16) + biases (f32) into
  two blob tensors; pad x and cast to bf16.
- Device (per core, 2 samples): sinc conv via im2col DMA (K=128+1 matmuls),
  SELU via exp/min decomposition: selu(x)/L = max(x,0) + min(A*e^x, A) - A
  (exp on ScalarE, rest on VectorE), abs+maxpool3, then the two conv encoders
  with spec/temp packed into M=128 matmuls, residual/downsample adds done as
  extra matmuls into the same PSUM accumulation group, avgpool as strided
  vector add (factor 1/2 folded into next weights), block3 means accumulated
  via accum_out, tiny GAT head + classifier fully on device.
"""
import math
import numpy as np
import ml_dtypes

import concourse.bass as bass
from concourse import bacc
import concourse.tile as tile
import concourse.mybir as mybir
from concourse.bass_utils import run_bass_kernel_spmd

BF16 = mybir.dt.bfloat16
F32 = mybir.dt.float32
AF = mybir.ActivationFunctionType
OP = mybir.AluOpType

LAM = 1.0507009873554805
ALPH = 1.6732632423543772
LNA = math.log(ALPH)

SR, K = 16000, 129
MIN_LOW, MIN_BAND = 50.0, 50.0
B, L0 = 16, 64512
L1, L2, L3 = 21504, 10752, 5376
PADL = 64 + L0 + 64  # 64640
NCORES = 8
BPC = B // NCORES  # 2 samples per core

NW = 41  # weight blob slots (128 cols each)
NB = 13  # bias blob slots (2 cols each)


# ----------------------------------------------------------------- host math
def _sinc_filters(low_hz, band_hz):
    low = MIN_LOW + np.abs(np.asarray(low_hz, np.float64))
    high = np.clip(low + MIN_BAND + np.abs(np.asarray(band_hz, np.float64)),
                   MIN_LOW, SR / 2)
    band = (high - low)[:, 0]
    n_ = (2 * np.pi * np.arange(-(K - 1) / 2.0, 0.0) / SR)[None, :]
    window = 0.54 - 0.46 * np.cos(2 * np.pi * np.linspace(0.0, K / 2 - 1, K // 2) / K)
    left = (np.sin(high @ n_) - np.sin(low @ n_)) / (n_ / 2) * window
    center = 2 * band[:, None]
    bp = np.concatenate([left, center, left[:, ::-1]], axis=1) / (2 * band[:, None])
    return bp.astype(np.float32)  # [70, 129]


def _fold_bn(w, b, bn):
    s = np.asarray(bn["g"], np.float32) / np.sqrt(np.asarray(bn["v"], np.float32) + 1e-5)
    w2 = np.asarray(w, np.float32) * s[:, None, None]
    b2 = (np.asarray(b, np.float32) - np.asarray(bn["m"], np.float32)) * s \
        + np.asarray(bn["b"], np.float32)
    return w2, b2


def _prep(params):
    """Returns (wblob bf16 [128,128*NW], bblob f32 [128,2*NB])."""
    p = params
    wb = np.zeros((128, 128 * NW), np.float32)
    bb = np.zeros((128, 2 * NB), np.float32)

    def putw(i, arr):  # arr [K, M] -> slot i
        k, m = arr.shape
        wb[:k, 128 * i:128 * i + m] = arr

    def putb(i, bias):  # plain col + (bias+lnA) col
        c = bias.shape[0]
        bb[:c, 2 * i] = bias
        bb[:c, 2 * i + 1] = bias + LNA

    filt = _sinc_filters(p["sinc"]["low_hz"], p["sinc"]["band_hz"])
    ws, bs = _fold_bn(filt[:, None, :], np.zeros(70, np.float32), p["sinc_bn"])
    ws = ws[:, 0, :]  # [70,129]
    putw(0, ws[:, :128].T)       # [128, 70]
    putw(1, ws[:, 128:129].T)    # [1, 70]
    putb(0, bs)

    def fold_enc(bi):
        """per-block folded params for both encoders."""
        out = []
        for e, enc in enumerate(("spec", "temp")):
            blk = p[enc][bi]
            w1, b1 = _fold_bn(blk["w1"], blk["b1"], blk["bn1"])
            w2, b2 = _fold_bn(blk["w2"], blk["b2"], blk["bn2"])
            d = {"w1": w1, "b1": b1, "w2": w2, "b2": b2}
            if "ds_w" in blk:
                dw, db = _fold_bn(blk["ds_w"], blk["ds_b"], blk["ds_bn"])
                d["dsw"], d["dsb"] = dw, db
            out.append(d)
        return out

    # block1: in h/L (70ch), packed M=128
    e0, e1 = fold_enc(0)
    for t in range(3):
        w = np.zeros((70, 128), np.float32)
        w[:, :64] = LAM * e0["w1"][:, :, t].T
        w[:, 64:] = LAM * e1["w1"][:, :, t].T
        putw(2 + t, w)
    putb(1, np.concatenate([e0["b1"], e1["b1"]]))
    for t in range(3):
        w = np.zeros((128, 128), np.float32)
        w[:64, :64] = LAM * e0["w2"][:, :, t].T
        w[64:, 64:] = LAM * e1["w2"][:, :, t].T
        putw(5 + t, w)
    dsw = np.zeros((70, 128), np.float32)
    dsw[:, :64] = LAM * e0["dsw"][:, :, 0].T
    dsw[:, 64:] = LAM * e1["dsw"][:, :, 0].T
    putw(8, dsw)
    putb(2, np.concatenate([e0["b2"] + e0["dsb"], e1["b2"] + e1["dsb"]]))

    # block2: in p1' (=2*avg/L), no ds; identity via (L/2)*I
    e0, e1 = fold_enc(1)
    for t in range(3):
        w = np.zeros((128, 128), np.float32)
        w[:64, :64] = (LAM / 2) * e0["w1"][:, :, t].T
        w[64:, 64:] = (LAM / 2) * e1["w1"][:, :, t].T
        putw(9 + t, w)
    putb(3, np.concatenate([e0["b1"], e1["b1"]]))
    for t in range(3):
        w = np.zeros((128, 128), np.float32)
        w[:64, :64] = LAM * e0["w2"][:, :, t].T
        w[64:, 64:] = LAM * e1["w2"][:, :, t].T
        putw(12 + t, w)
    putw(15, (LAM / 2) * np.eye(128, dtype=np.float32))
    putb(4, np.concatenate([e0["b2"], e1["b2"]]))

    # block3: per-enc (M=128 each), zero-padded K=128 lhsT
    encs = fold_enc(2)
    for e, d in enumerate(encs):
        r0 = 64 * e
        for t in range(3):
            w = np.zeros((128, 128), np.float32)
            w[r0:r0 + 64, :] = (LAM / 2) * d["w1"][:, :, t].T
            putw(16 + 3 * e + t, w)
        putb(7 + e, d["b1"])
        for t in range(3):
            putw(22 + 3 * e + t, LAM * d["w2"][:, :, t].T)
        w = np.zeros((128, 128), np.float32)
        w[r0:r0 + 64, :] = (LAM / 2) * d["dsw"][:, :, 0].T
        putw(28 + e, w)
        putb(5 + e, d["b2"] + d["dsb"])

    # gat1: fold lambda/L3 (mean + final selu scale) into fc_w
    g1 = p["gat1"]
    putw(30, (LAM / L3) * np.asarray(g1["fc_w"], np.float32))  # [128,64]
    bb[:64, 18] = np.asarray(g1["fc_b"], np.float32)
    putw(31, np.asarray(g1["a_w"], np.float32)[:64, :])   # [64,1]
    putw(32, np.asarray(g1["a_w"], np.float32)[64:, :])
    g2 = p["gat2"]
    putw(33, np.asarray(g2["fc_w"], np.float32))  # [64,32]
    bb[:32, 20] = np.asarray(g2["fc_b"], np.float32)
    putw(34, np.asarray(g2["a_w"], np.float32)[:32, :])
    putw(35, np.asarray(g2["a_w"], np.float32)[32:, :])
    putw(36, np.ones((1, 128), np.float32))
    E = np.zeros((32, 64), np.float32); E[:, :32] = np.eye(32)
    putw(37, E)
    E = np.zeros((32, 64), np.float32); E[:, 32:] = np.eye(32)
    putw(38, E)
    c = p["cls"]
    putw(39, np.asarray(c["w1"], np.float32))  # [64,32]
    putb(11, np.asarray(c["b1"], np.float32))
    putw(40, LAM * np.asarray(c["w2"], np.float32))  # [32,2]
    bb[:2, 24] = np.asarray(c["b2"], np.float32)
    ab1 = float(np.asarray(g1["a_b"]).reshape(-1)[0])
    ab2 = float(np.asarray(g2["a_b"]).reshape(-1)[0])
    return (wb.astype(ml_dtypes.bfloat16), bb.astype(np.float32), ab1, ab2)


# ------------------------------------------------------------- device kernel
def _sv(base_full, c0, step, count):
    """strided free-dim view of an SBUF tile AP starting at col c0."""
    b = base_full[:, c0:c0 + 1]
    return bass.AP(tensor=b.tensor, offset=b.offset, ap=[b.ap[0], [step, count]])


def _rep(base_full, c0, ostep, ocnt, icnt):
    """cols like {c0, c0, c0+ostep, c0+ostep} view: outer ostep x ocnt, inner repeat icnt."""
    b = base_full[:, c0:c0 + 1]
    return bass.AP(tensor=b.tensor, offset=b.offset,
                   ap=[b.ap[0], [ostep, ocnt], [0, icnt]])


def build_nc(ab1, ab2):
    nc = bacc.Bacc()
    x_ext = nc.declare_dram_parameter("x", [BPC, PADL], BF16, isOutput=False)
    wb_ext = nc.declare_dram_parameter("wblob", [128, 128 * NW], BF16, isOutput=False)
    bb_ext = nc.declare_dram_parameter("bblob", [128, 2 * NB], F32, isOutput=False)
    out_ext = nc.declare_dram_parameter("out", [2, BPC], F32, isOutput=True)

    with tile.TileContext(nc) as tc:
        with (
            tc.tile_pool(name="wp", bufs=1) as wp,
            tc.tile_pool(name="ap_", bufs=1) as apool,
            tc.tile_pool(name="sp", bufs=2) as sp,
            tc.tile_pool(name="pp", bufs=2, space="PSUM") as pp,
        ):
            WB = wp.tile([128, 128 * NW], BF16, tag="wb", name="wb")
            nc.sync.dma_start(out=WB, in_=wb_ext[:])
            BBt = wp.tile([128, 2 * NB], F32, tag="bb", name="bb")
            nc.sync.dma_start(out=BBt, in_=bb_ext[:])

            def W(i, k, m):
                return WB[0:k, 128 * i:128 * i + m]

            def Bp(i, c):
                return BBt[0:c, 2 * i:2 * i + 1]

            def Be(i, c):
                return BBt[0:c, 2 * i + 1:2 * i + 2]

            nodes = wp.tile([128, 4], F32, tag="nodes", name="nodes")

            ev_ctr = [0]

            def evict(ps_ap, c, w, bi, out_ap, accum=None):
                """psum [c,w] -> out bf16: selu(x)/LAM with bias from slot bi."""
                t = sp.tile([128, 1536], BF16, tag="ev_t", name="ev_t")[:c, :w]
                r = sp.tile([128, 1536], BF16, tag="ev_r", name="ev_r")[:c, :w]
                nc.scalar.activation(t, ps_ap, AF.Exp, bias=Be(bi, c), scale=1.0)
                if ev_ctr[0] % 2 == 0:
                    nc.scalar.activation(r, ps_ap, AF.Relu, bias=Bp(bi, c), scale=1.0)
                else:
                    nc.vector.tensor_scalar(r, ps_ap, Bp(bi, c), 0.0, OP.add, OP.max)
                ev_ctr[0] += 1
                nc.vector.tensor_scalar(t, t, float(ALPH), None, OP.min)
                nc.vector.scalar_tensor_tensor(
                    out_ap, t, float(-ALPH), r, OP.add, OP.add, accum_out=accum)

            def conv_chunks(in_buf, cin, L, taps, bi, out_buf, extra=None,
                            means_col=None):
                """3-tap conv (+optional extra (wslot, buf, cin2) 1x1-matmul)
                over length L; writes selu into out_buf cols [1, L+1]
                or accumulates means into nodes[:, means_col]."""
                if means_col is not None:
                    parts = apool.tile([128, 4], F32, tag="pt", name="pt")
                nchunk = 0
                for c0 in range(0, L, 1536):
                    wdt = min(1536, L - c0)
                    ps = pp.tile([128, 1536], F32, tag="ps", name="ps")[:, :wdt]
                    for j0 in range(0, wdt, 512):
                        nj = min(512, wdt - j0)
                        pj = ps[:, j0:j0 + nj]
                        for t in range(3):
                            nc.tensor.matmul(
                                pj, W(taps[t], cin, 128),
                                in_buf[0:cin, c0 + j0 + t:c0 + j0 + t + nj],
                                start=(t == 0), stop=(t == 2 and extra is None))
                        if extra is not None:
                            ws, ebuf, cin2 = extra
                            nc.tensor.matmul(
                                pj, W(ws, cin2, 128),
                                ebuf[0:cin2, c0 + j0 + 1:c0 + j0 + 1 + nj],
                                start=False, stop=True)
                    if means_col is None:
                        evict(ps, 128, wdt, bi, out_buf[:, 1 + c0:1 + c0 + wdt])
                    else:
                        y3 = sp.tile([128, 1536], BF16, tag="y3", name="y3")[:, :wdt]
                        evict(ps, 128, wdt, bi, y3,
                              accum=parts[:, nchunk:nchunk + 1])
                    nchunk += 1
                if means_col is not None:
                    nc.vector.tensor_reduce(
                        nodes[:, means_col:means_col + 1], parts[:, :nchunk],
                        axis=mybir.AxisListType.X, op=OP.add)

            for s in range(BPC):
                # ---------------- sinc + maxpool3 -> h [70, L1]
                h = apool.tile([70, L1 + 2], BF16, tag="A", name="A")
                nc.vector.memset(h[:, 0:1], 0.0)
                nc.vector.memset(h[:, L1 + 1:L1 + 2], 0.0)
                for g in range(L0 // 1536):
                    im = sp.tile([128, 1536], BF16, tag="im", name="im")
                    xb = x_ext[s:s + 1, g * 1536:g * 1536 + 1]
                    nc.gpsimd.dma_start(out=im, in_=bass.AP(
                        tensor=xb.tensor, offset=xb.offset,
                        ap=[[1, 128], [1, 1536]]))
                    im1 = sp.tile([1, 1536], BF16, tag="im1", name="im1")
                    xb1 = x_ext[s:s + 1, g * 1536 + 128:g * 1536 + 129]
                    nc.gpsimd.dma_start(out=im1, in_=bass.AP(
                        tensor=xb1.tensor, offset=xb1.offset,
                        ap=[[1, 1], [1, 1536]]))
                    ps = pp.tile([128, 1536], F32, tag="ps", name="ps")[:70, :]
                    for j in range(3):
                        pj = ps[:, 512 * j:512 * (j + 1)]
                        nc.tensor.matmul(pj, W(0, 128, 70),
                                         im[0:128, 512 * j:512 * j + 512],
                                         start=True, stop=False)
                        nc.tensor.matmul(pj, W(1, 1, 70),
                                         im1[0:1, 512 * j:512 * j + 512],
                                         start=False, stop=True)
                    y = sp.tile([70, 1536], BF16, tag="sy", name="sy")
                    evict(ps, 70, 1536, 0, y)
                    a = sp.tile([70, 1536], BF16, tag="sa", name="sa")
                    nc.vector.scalar_tensor_tensor(a, y, -1.0, y, OP.mult, OP.max)
                    m = sp.tile([70, 512], BF16, tag="sm", name="sm")
                    nc.vector.tensor_tensor(m, _sv(a, 0, 3, 512), _sv(a, 1, 3, 512),
                                            OP.max)
                    nc.vector.tensor_tensor(h[:, 1 + 512 * g:513 + 512 * g], m,
                                            _sv(a, 2, 3, 512), OP.max)

                # ---------------- block1
                t1 = apool.tile([128, L1 + 2], BF16, tag="B", name="B")
                nc.vector.memset(t1[:, 0:1], 0.0)
                nc.vector.memset(t1[:, L1 + 1:L1 + 2], 0.0)
                conv_chunks(h, 70, L1, (2, 3, 4), 1, t1)
                y1 = apool.tile([128, L1 + 2], BF16, tag="C", name="C")
                conv_chunks(t1, 128, L1, (5, 6, 7), 2, y1, extra=(8, h, 70))
                p1 = apool.tile([128, L2 + 2], BF16, tag="B", name="B")
                nc.vector.memset(p1[:, 0:1], 0.0)
                nc.vector.memset(p1[:, L2 + 1:L2 + 2], 0.0)
                nc.vector.tensor_tensor(p1[:, 1:L2 + 1], _sv(y1, 1, 2, L2),
                                        _sv(y1, 2, 2, L2), OP.add)

                # ---------------- block2
                t2 = apool.tile([128, L2 + 2], BF16, tag="A", name="A")
                nc.vector.memset(t2[:, 0:1], 0.0)
                nc.vector.memset(t2[:, L2 + 1:L2 + 2], 0.0)
                conv_chunks(p1, 128, L2, (9, 10, 11), 3, t2)
                y2 = apool.tile([128, L2 + 2], BF16, tag="C", name="C")
                conv_chunks(t2, 128, L2, (12, 13, 14), 4, y2, extra=(15, p1, 128))
                p2 = apool.tile([128, L3 + 2], BF16, tag="D", name="D")
                nc.vector.memset(p2[:, 0:1], 0.0)
                nc.vector.memset(p2[:, L3 + 1:L3 + 2], 0.0)
                nc.vector.tensor_tensor(p2[:, 1:L3 + 1], _sv(y2, 1, 2, L3),
                                        _sv(y2, 2, 2, L3), OP.add)

                # ---------------- block3 per encoder -> node means
                for e in range(2):
                    t3 = apool.tile([128, L3 + 2], BF16, tag="E", name="E")
                    nc.vector.memset(t3[:, 0:1], 0.0)
                    nc.vector.memset(t3[:, L3 + 1:L3 + 2], 0.0)
                    conv_chunks(p2, 128, L3, (16 + 3 * e, 17 + 3 * e, 18 + 3 * e),
                                7 + e, t3)
                    conv_chunks(t3, 128, L3, (22 + 3 * e, 23 + 3 * e, 24 + 3 * e),
                                5 + e, None, extra=(28 + e, p2, 128),
                                means_col=2 * s + e)

            # ---------------- GAT head + classifier
            nodes_b = wp.tile([128, 4], BF16, tag="nodesb", name="nodesb")
            nc.vector.tensor_copy(nodes_b, nodes)

            def gat(inp_b, din, dout, wfc, bfc, wa1, wa2, ab):
                ps = pp.tile([128, 512], F32, tag="gps", name="gps")[:dout, :4]
                nc.tensor.matmul(ps, W(wfc, din, dout), inp_b[0:din, 0:4],
                                 start=True, stop=True)
                h1 = wp.tile([dout, 4], F32, tag=f"h1_{dout}", name=f"h1_{dout}")
                nc.vector.tensor_scalar(h1, ps, Bp(bfc, dout), None, OP.add)
                h1b = wp.tile([dout, 4], BF16, tag=f"h1b_{dout}", name=f"h1b_{dout}")
                nc.vector.tensor_copy(h1b, h1)
                sij = wp.tile([1, 8], F32, tag=f"sij_{dout}", name=f"sij_{dout}")
                for k, wa in enumerate((wa1, wa2)):
                    ps2 = pp.tile([128, 512], F32, tag="gps", name="gps")[:1, :4]
                    nc.tensor.matmul(ps2, W(wa, dout, 1), h1b[0:dout, 0:4],
                                     start=True, stop=True)
                    nc.vector.tensor_copy(sij[:, 4 * k:4 * k + 4], ps2)
                z0 = wp.tile([1, 4], F32, tag=f"z0_{dout}", name=f"z0_{dout}")
                z1 = wp.tile([1, 4], F32, tag=f"z1_{dout}", name=f"z1_{dout}")
                nc.vector.scalar_tensor_tensor(
                    z0, sij[:, 0:4], float(ab), _rep(sij, 4, 2, 2, 2),
                    OP.add, OP.add)
                nc.vector.scalar_tensor_tensor(
                    z1, sij[:, 0:4], float(ab), _rep(sij, 5, 2, 2, 2),
                    OP.add, OP.add)
                zl0 = wp.tile([1, 4], F32, tag=f"zl0_{dout}", name=f"zl0_{dout}")
                zl1 = wp.tile([1, 4], F32, tag=f"zl1_{dout}", name=f"zl1_{dout}")
                nc.vector.scalar_tensor_tensor(zl0, z0, 0.2, z0, OP.mult, OP.max)
                nc.vector.scalar_tensor_tensor(zl1, z1, 0.2, z1, OP.mult, OP.max)
                d = wp.tile([1, 4], BF16, tag=f"d_{dout}", name=f"d_{dout}")
                nc.vector.tensor_tensor(d, zl0, zl1, OP.subtract)
                a0 = wp.tile([1, 4], BF16, tag=f"a0_{dout}", name=f"a0_{dout}")
                nc.scalar.activation(a0, d, AF.Sigmoid, scale=0.5)
                psb = pp.tile([128, 512], F32, tag="gps", name="gps")[:dout, :4]
                nc.tensor.matmul(psb, W(36, 1, dout), a0[0:1, 0:4],
                                 start=True, stop=True)
                diff = wp.tile([dout, 2], F32, tag=f"df_{dout}", name=f"df_{dout}")
                nc.vector.tensor_tensor(diff, _sv(h1, 0, 2, 2), _sv(h1, 1, 2, 2),
                                        OP.subtract)
                m1 = wp.tile([dout, 4], F32, tag=f"m1_{dout}", name=f"m1_{dout}")
                nc.vector.scalar_tensor_tensor(m1, psb, 1.0,
                                               _rep(diff, 0, 1, 2, 2),
                                               OP.mult, OP.mult)
                h2p = wp.tile([dout, 4], F32, tag=f"h2p_{dout}", name=f"h2p_{dout}")
                nc.vector.tensor_tensor(h2p, m1, _rep(h1, 1, 2, 2, 2), OP.add)
                te = wp.tile([dout, 4], F32, tag=f"te_{dout}", name=f"te_{dout}")
                nc.scalar.activation(te, h2p, AF.Exp)
                r2 = wp.tile([dout, 4], F32, tag=f"r2_{dout}", name=f"r2_{dout}")
                nc.vector.tensor_scalar(r2, h2p, 0.0, None, OP.max)
                ve = wp.tile([dout, 4], F32, tag=f"ve_{dout}", name=f"ve_{dout}")
                nc.vector.tensor_scalar(ve, te, 1.0, None, OP.min)
                h2 = wp.tile([dout, 4], BF16, tag=f"h2_{dout}", name=f"h2_{dout}")
                nc.vector.scalar_tensor_tensor(h2, ve, -1.0, r2, OP.add, OP.add)
                return h2

            h2 = gat(nodes_b, 128, 64, 30, 9, 31, 32, ab1)
            h4 = gat(h2, 64, 32, 33, 10, 34, 35, ab2)

            fps = pp.tile([128, 512], F32, tag="gps", name="gps")[:64, :2]
            nc.tensor.matmul(fps, W(37, 32, 64), _sv(h4, 0, 2, 2),
                             start=True, stop=False)
            nc.tensor.matmul(fps, W(38, 32, 64), _sv(h4, 1, 2, 2),
                             start=False, stop=True)
            flatb = wp.tile([64, 2], BF16, tag="flatb", name="flatb")
            nc.vector.tensor_copy(flatb, fps)
            yps = pp.tile([128, 512], F32, tag="gps", name="gps")[:32, :2]
            nc.tensor.matmul(yps, W(39, 64, 32), flatb[0:64, 0:2],
                             start=True, stop=True)
            tc_ = wp.tile([32, 2], BF16, tag="ct", name="ct")
            rc = wp.tile([32, 2], BF16, tag="cr", name="cr")
            nc.scalar.activation(tc_, yps, AF.Exp, bias=Be(11, 32), scale=1.0)
            nc.vector.tensor_scalar(rc, yps, Bp(11, 32), 0.0, OP.add, OP.max)
            vc = wp.tile([32, 2], BF16, tag="cv", name="cv")
            nc.vector.tensor_scalar(vc, tc_, float(ALPH), None, OP.min)
            y1c = wp.tile([32, 2], BF16, tag="cy", name="cy")
            nc.vector.scalar_tensor_tensor(y1c, vc, float(-ALPH), rc, OP.add, OP.add)
            ops = pp.tile([128, 512], F32, tag="gps", name="gps")[:2, :2]
            nc.tensor.matmul(ops, W(40, 32, 2), y1c[0:32, 0:2],
                             start=True, stop=True)
            osb = wp.tile([2, BPC], F32, tag="osb", name="osb")
            nc.vector.tensor_scalar(osb, ops, Bp(12, 2), None, OP.add)
            nc.sync.dma_start(out=out_ext[:], in_=osb)
    nc.compile()
    return nc


# ----------------------------------------------------------------- interface
_CACHE = {}


def _tree_np(obj):
    if isinstance(obj, dict):
        return {k: _tree_np(v) for k, v in obj.items()}
    if isinstance(obj, (list, tuple)):
        return [_tree_np(v) for v in obj]
    return np.asarray(obj)


def run(x, params, trace=False):
    x = np.asarray(x, np.float32)
    params = _tree_np(params)
    wb, bb, ab1, ab2 = _prep(params)
    key = "nc"
    if key not in _CACHE:
        _CACHE[key] = build_nc(ab1, ab2)
    nc = _CACHE[key]
    xpad = np.pad(x, ((0, 0), (64, 64))).astype(ml_dtypes.bfloat16)
    in_maps = []
    for c in range(NCORES):
        in_maps.append({
            "x": xpad[BPC * c:BPC * (c + 1)],
            "wblob": wb,
            "bblob": bb,
        })
    res = run_bass_kernel_spmd(nc, in_maps, core_ids=list(range(NCORES)),
                               trace=trace)
    outs = [np.asarray(res.results[c]["out"], np.float32).T for c in range(NCORES)]
    return np.concatenate(outs, 0), res


def kernel(**inputs):
    out, _ = run(inputs["x"], inputs["params"])
    return out
